# revision 82
# baseline (speedup 1.0000x reference)
"""Trainium2 Bass kernel for nn_Attention (B=64, N=289, C=768, H=12).

Data-parallel over batch: 8 batches per NeuronCore x 8 cores, no collectives.

Per-core pipeline (all matmuls bf16 with f32 PSUM accumulation):
  1. qkv GEMM token-major.  Mean-centering of q/k is folded into the
     weights host-side (W' = W - per-head row mean, exact), so the GEMM
     directly emits zero-mean q/k and no mean stats are needed.  v heads
     are computed in the same per-block pass and scattered into per-batch
     token-major v65 tiles via SBUF->SBUF DMA (arbitrary partition shift).
  2. q/k head-wise layernorm reduces to one rsqrt scale: var = sumsq/64,
     rsqrt via DVE bit-trick + Newton, z = q * s.
  3. rope fused with the LN gain/bias via host-precomputed tables; even
     lanes on DVE, odd lanes on GPSIMD.  1/sqrt(D) folded into q tables.
  4. q/k -> feature-major strips via XBAR DMA transpose ([128,128] bf16
     SBUF->SBUF), split across the two HWDGE engines (k on SP, q on ACT)
     so the two transpose pipes run in parallel.
  5. scores computed transposed ST[k_tok, q_tok] (softmax without max
     subtraction -- LN-bounded logits), exp on ScalarE straight from PSUM.
  6. AV with a ones-column appended to V: OT[65, q] where row 64 carries
     the softmax sums; sum rows gathered via aligned ScalarE copies into
     stride-32 partitions + 3 partition-matched DMAs on the ACT queue.
     The normalize tail (reciprocal, broadcast, multiply) is deferred and
     emitted interleaved with the NEXT batch's scores so the PE never
     waits on it.
  7. proj GEMM feature-major + bias, bf16 out DMA as yT[768, 2312]; the
     host transposes back and casts to f32.
"""

import sys

if "/opt/trn_rl_repo" not in sys.path:
    sys.path.insert(0, "/opt/trn_rl_repo")

from contextlib import ExitStack

import ml_dtypes
import numpy as np

import concourse.bass as bass
import concourse.tile as tile
from concourse import bacc, mybir
from concourse.bass_utils import run_bass_kernel_spmd

F32 = mybir.dt.float32
BF16 = mybir.dt.bfloat16
I32 = mybir.dt.int32
BF = ml_dtypes.bfloat16
OP = mybir.AluOpType
AF = mybir.ActivationFunctionType

B, N, C, H = 64, 289, 768, 12
D = C // H  # 64
NCORES = 8
BPC = B // NCORES  # 8 batches per core
T = BPC * N  # 2312 tokens per core
NT = (T + 127) // 128  # 19 token blocks
TPAD = NT * 128  # 2432
EPS = 1e-5
CHW = 6 * D  # 384 columns per qkv chunk (6 heads; centering is in W)
MAGIC = 0x5F3759DF

_CACHE = {}


def _batch_blocks():
    out, r = [], 0
    while r < N:
        rows = min(128, N - r)
        out.append((r, rows))
        r += rows
    return out


def _block_pieces(i):
    """Split global token block i into per-(batch, kblock) pieces.

    Returns [(src_row, b, kb, dst_row, take)].
    """
    pieces = []
    g = 128 * i
    end = min(128 * (i + 1), T)
    while g < end:
        b = g // N
        l = g - N * b
        kb = l // 128
        row0 = l - 128 * kb
        take = min(end, N * (b + 1), N * b + 128 * (kb + 1)) - g
        pieces.append((g - 128 * i, b, kb, row0, take))
        g += take
    return pieces


def _build_program():
    nc = bacc.Bacc("TRN2", target_bir_lowering=False, debug=False,
                   num_devices=NCORES)

    # block-major x: xb[i, p, c*128+t] = x^T[c*128+p, i*128+t] so each
    # 128-token block loads as one 1536B-contiguous descriptor per partition
    xb = nc.dram_tensor("xb", [NT, 128, 6 * 128], BF16,
                        kind="ExternalInput").ap()
    wqkvT = nc.dram_tensor("wqkvT", [C, 6 * CHW], BF16,
                           kind="ExternalInput").ap()
    wprojT = nc.dram_tensor("wprojT", [C, C], BF16, kind="ExternalInput").ap()
    pbias = nc.dram_tensor("pbias", [C], F32, kind="ExternalInput").ap()
    # fused rope+LN tables, order: qCT, qST, qB2, kCT, kST, kB2
    tabs = nc.dram_tensor("tabs", [6, TPAD, D], BF16,
                          kind="ExternalInput").ap()
    sel = nc.dram_tensor("sel", [12, C], BF16, kind="ExternalInput").ap()
    # batch-contiguous out: out[b, p, co, n] = y[b, n, co*128+p]; one
    # 3468B descriptor per partition per batch
    out = nc.dram_tensor("out", [BPC, 128, 6, N], BF16,
                         kind="ExternalOutput").ap()

    with tile.TileContext(nc) as tc, ExitStack() as ctx:
        consts = ctx.enter_context(tc.tile_pool(name="consts", bufs=1))
        work = ctx.enter_context(tc.tile_pool(name="work", bufs=2))
        blk = ctx.enter_context(tc.tile_pool(name="blk", bufs=2))
        shal = ctx.enter_context(tc.tile_pool(name="shal", bufs=2))
        rope_scr = ctx.enter_context(tc.tile_pool(name="rope", bufs=2))
        strips = ctx.enter_context(tc.tile_pool(name="strips", bufs=1))
        vpool = ctx.enter_context(tc.tile_pool(name="vpool", bufs=15))
        ptpool = ctx.enter_context(tc.tile_pool(name="ptpool", bufs=7))
        otsbp = ctx.enter_context(tc.tile_pool(name="otsbp", bufs=10))
        otnpool = ctx.enter_context(tc.tile_pool(name="otnpool", bufs=9))
        bpool = ctx.enter_context(tc.tile_pool(name="bpool", bufs=2))
        mmps = ctx.enter_context(tc.tile_pool(name="mmps", bufs=2,
                                              space="PSUM"))
        scps = ctx.enter_context(tc.tile_pool(name="scps", bufs=2,
                                              space="PSUM"))
        otps = ctx.enter_context(tc.tile_pool(name="otps", bufs=2,
                                              space="PSUM"))

        # ---- persistent constants ----
        # xg block 0/1 prefetches go first on SP (see xgpool below), then
        # wq (block-0 matmuls pace with their arrival); tabs first on ACT
        # (block-0 rope needs them), then wp/sel/pbias (phase B only).
        xgpool = ctx.enter_context(tc.tile_pool(name="xgpool", bufs=3))
        xg_tiles = {}

        def fetch_xg(i):
            if i < NT and i not in xg_tiles:
                t = xgpool.tile([128, 6, 128], BF16, tag="xg")
                nc.scalar.dma_start(
                    t[:], xb[i].rearrange("p (c t) -> p c t", t=128))
                xg_tiles[i] = t
            return xg_tiles.get(i)

        fetch_xg(0)
        fetch_xg(1)
        # DMA queues fair-share bandwidth, so anything issued at t=0
        # delays the critical wq+xg path.  Only wq goes out up front;
        # tabs are issued after block 0 and wp/sel/pbias after block 2
        # (see emit_tile) -- by then the startup burst has drained.
        wq = []
        for c in range(6):
            t = consts.tile([128, 6 * CHW], BF16, tag=f"wq{c}")
            dq = nc.sync if c % 2 == 0 else nc.scalar
            dq.dma_start(t[:], wqkvT[c * 128:(c + 1) * 128, :])
            wq.append(t)
        tab_t = [consts.tile([128, NT, D], BF16, tag=f"tab{k}",
                             name=f"tab{k}") for k in range(6)]
        wp = [consts.tile([128, C], BF16, tag=f"wp{c}", name=f"wp{c}")
              for c in range(6)]
        pbias_t = consts.tile([128, 6], F32, tag="pbias")
        sel_t = consts.tile([12, C], BF16, tag="sel")
        magic_t = consts.tile([128, 24], I32, tag="magic")
        nc.vector.memset(magic_t[:], MAGIC)

        def load_tabs():
            # emitted between block-0's evacs and its rope: the ACT
            # sequencer issues these only once the evacs retire, by which
            # time the wq burst has mostly drained
            for k in range(6):
                nc.scalar.dma_start(
                    tab_t[k][:],
                    tabs[k].rearrange("(i p) d -> p i d", p=128))

        def load_phaseb_consts():
            for c in range(6):
                nc.scalar.dma_start(
                    wp[c][:], wprojT[c * 128:(c + 1) * 128, :])
            nc.scalar.dma_start(
                pbias_t[:], pbias.rearrange("(a p) -> p a", p=128))
            nc.scalar.dma_start(sel_t[:], sel)

        # q/k feature-major strips, all head pairs in one tile: strip p
        # lives at [:, p, :].  One XBAR DMA transpose per (block, half)
        # fills all six strips (out[:, j, :] = in[:, 128j:128j+128].T).
        qT = strips.tile([128, 6, TPAD], BF16, tag="qT", name="qT")
        kT = strips.tile([128, 6, TPAD], BF16, tag="kT", name="kT")

        kblocks = _batch_blocks()
        v65 = {}  # (b, kb) -> tile [128, H, D+1]

        def get_v65(b, kb):
            if (b, kb) not in v65:
                # ones column arrives via the vsb scatter; no memset needed
                t = vpool.tile([128, H, D + 1], BF16, tag="v65")
                v65[(b, kb)] = t
            return v65[(b, kb)]

        # ---- phase A: qkv GEMM + LN + rope + DMA transpose, per 128-blk.
        # Split into start / per-chunk-pair matmuls / post so phase-B units
        # can zipper between the matmul groups, keeping the PE stream
        # gapless (the p-state ramp only holds at 2.4GHz while busy).
        def start_tile(i):
            xg = fetch_xg(i)
            fetch_xg(i + 2)  # prefetch ahead of this block's transposes
            xg_tiles.pop(i, None)
            sumsq = work.tile([128, 24], F32, tag="sumsq")
            qsb = blk.tile([128, H, D], BF16, tag="qsb")
            ksb = blk.tile([128, H, D], BF16, tag="ksb")
            # vsb [128, H, 65]: ones column baked in so scatter rows are
            # contiguous
            vsb = blk.tile([128, H, D + 1], BF16, tag="vsb")
            nc.vector.memset(vsb[:, :, D:D + 1], 1.0)
            return dict(i=i, xg=xg, sumsq=sumsq, qsb=qsb, ksb=ksb, vsb=vsb)

        def tile_mm(st, js):
            # qkv GEMM chunks; evacuation trails on ScalarE (the DVE queue
            # is busy with the rope chain, and mmps recycling gates the
            # next matmul group)
            xg, qsb, ksb, vsb = st["xg"], st["qsb"], st["ksb"], st["vsb"]
            halves = (qsb, ksb)
            for j in js:
                ps = mmps.tile([128, CHW], F32, tag="mm")
                for c in range(6):
                    nc.tensor.matmul(ps[:], lhsT=xg[:, c, :],
                                     rhs=wq[c][:, j * CHW:(j + 1) * CHW],
                                     start=(c == 0), stop=(c == 5))
                if j < 4:
                    dst = halves[j // 2]
                    jl = j % 2
                    nc.scalar.copy(
                        dst[:, jl * 6:(jl + 1) * 6, :].rearrange(
                            "p h d -> p (h d)"), ps[:])
                else:
                    nc.scalar.copy(
                        vsb[:, (j - 4) * 6:(j - 3) * 6, :D],
                        ps[:].rearrange("p (h d) -> p h d", d=D))

        def tile_post(st):
            i = st["i"]
            sumsq, vsb = st["sumsq"], st["vsb"]
            halves = (st["qsb"], st["ksb"])
            if i == 0:
                load_tabs()
            elif i == 3:
                load_phaseb_consts()

            # squares + per-head reduces (ScalarE + DVE); the square scratch
            # borrows the m1 rope tag (freed by its reduce before the rope
            # chain reuses the buffer)
            for half in range(2):
                sq = rope_scr.tile([128, H, D], BF16, tag="m1")
                nc.scalar.activation(sq[:], halves[half][:], AF.Square)
                nc.vector.tensor_reduce(
                    out=sumsq[:, half * 12:(half + 1) * 12], in_=sq[:],
                    axis=mybir.AxisListType.X, op=OP.add)

            # one consolidated stats chain on [128, 24]:
            # s = rsqrt(sumsq/64 + eps)  (q/k are zero-mean by construction)
            vpe = shal.tile([128, 24], F32, tag="vpe")
            srs = work.tile([128, 24], F32, tag="srs")
            nt1 = shal.tile([128, 24], F32, tag="nt1")
            nc.vector.tensor_scalar(out=vpe[:], in0=sumsq[:],
                                    scalar1=1.0 / D, scalar2=float(EPS),
                                    op0=OP.mult, op1=OP.add)
            nc.vector.tensor_scalar(out=srs[:].bitcast(I32),
                                    in0=vpe[:].bitcast(I32), scalar1=1,
                                    scalar2=None,
                                    op0=OP.logical_shift_right)
            nc.vector.tensor_tensor(out=srs[:].bitcast(I32),
                                    in0=magic_t[:],
                                    in1=srs[:].bitcast(I32),
                                    op=OP.subtract)
            # Newton: y *= 1.5 - 0.5*x*y^2
            nc.vector.tensor_mul(nt1[:], srs[:], srs[:])
            nc.vector.tensor_mul(nt1[:], nt1[:], vpe[:])
            nc.vector.tensor_scalar(out=nt1[:], in0=nt1[:],
                                    scalar1=-0.5, scalar2=1.5,
                                    op0=OP.mult, op1=OP.add)
            nc.vector.tensor_mul(srs[:], srs[:], nt1[:])
            # bf16 copy: an f32 broadcast operand halves DVE throughput
            srsb = work.tile([128, 24], BF16, tag="srsb")
            nc.vector.tensor_copy(srsb[:], srs[:])

            for half, (tb, rot_tag, dst, dq) in enumerate(
                    ((0, "qrot", qT, nc.sync), (3, "krot", kT, nc.sync))):
                hsb = halves[half]
                hsrs = srsb[:, half * 12:(half + 1) * 12]
                z = blk.tile([128, H, D], BF16, tag="z" + rot_tag)
                # z = q * s with s broadcast along d (q is zero-mean)
                nc.vector.tensor_tensor(
                    out=z[:], in0=hsb[:],
                    in1=hsrs[:, :, None].broadcast_to([128, H, D]),
                    op=OP.mult)

                # rope (+ folded gain/bias), deinterleaved pair layout:
                # head cols [0:32] = even lanes, [32:64] = odd lanes.
                # All DVE (GpSimd concurrency poisons DVE throughput ~4x).
                # m1 = z*CT; m2 = z*STs2 with STs2 = [ge*s | -go*s], so the
                # cross-half combine is two plain-slice adds (no swap AP):
                #   rot_e = m1_e + m2_o,  rot_o = m1_o + m2_e
                rot = blk.tile([128, H, D], BF16, tag=rot_tag)
                CT = tab_t[tb][:, i, :]
                ST = tab_t[tb + 1][:, i, :]
                B2 = tab_t[tb + 2][:, i, :]

                def bcf(ap):
                    return ap[:, None, :].broadcast_to([128, H, D])

                # no in-place ops: DVE runs read-modify-write at half rate
                m1 = rope_scr.tile([128, H, D], BF16, tag="m1")
                m2 = rope_scr.tile([128, H, D], BF16, tag="m2")
                t3 = rope_scr.tile([128, H, D], BF16, tag="t3")
                nc.vector.tensor_mul(m1[:], z[:], bcf(CT))
                nc.vector.tensor_mul(m2[:], z[:], bcf(ST))
                nc.vector.tensor_add(t3[:, :, 0:32], m1[:, :, 0:32],
                                     m2[:, :, 32:64])
                nc.vector.tensor_add(t3[:, :, 32:64], m1[:, :, 32:64],
                                     m2[:, :, 0:32])
                nc.vector.tensor_add(rot[:], t3[:], bcf(B2))

                # one XBAR DMA transpose fills all 6 strips of this block
                dq.dma_start_transpose(
                    dst[:, :, i * 128:(i + 1) * 128],
                    rot[:].rearrange("p h d -> p (h d)"))

            # v scatter last: vsb is long done by the time SP reaches these
            for (src_row, b, kb, dst_row, take) in _block_pieces(i):
                vt = get_v65(b, kb)
                nc.sync.dma_start(
                    vt[dst_row:dst_row + take, :, :],
                    vsb[src_row:src_row + take, :, :])

        # ---- phase B: per batch attention + proj ----
        pending_proj = []

        def norm_pair(b, c):
            # normalize strips 2c, 2c+1 of batch b: their reciprocal rows
            # (rinvb[4c:4c+4]) were produced incrementally by gather_chunk
            st_ = batch_state[b]
            for p in (2 * c, 2 * c + 1):
                o = otnpool.tile([128, N], BF16, tag="otn")
                # one matmul broadcasts both heads' reciprocal rows
                # across the pair's 128 partitions; lives in scps so
                # the AV ot ring never waits on the otn chain
                rb = scps.tile([128, 2, 512], F32, tag="sc")
                nc.tensor.matmul(rb[:, 0, :N],
                                 lhsT=sel_t[:, p * 128:(p + 1) * 128],
                                 rhs=st_["rinvb"][:], start=True, stop=True)
                nc.vector.tensor_mul(o[:], st_["otsb"][p][:], rb[:, 0, :N])
                st_["otn"][p] = o

        def gather_chunk(b, c):
            # sums rows for strips 2c, 2c+1 -> reciprocal -> bf16.
            # Engine ops need mod-32 partition bases, so each chunk works
            # in its own base-0 tile and a small DMA places the result at
            # rinvb rows 4c..4c+4.
            st_ = batch_state[b]
            r4 = st_["rowbuf"][:].rearrange("(a b) (c n) -> a b c n",
                                            b=32, n=N)
            sumc = bpool.tile([4, N], F32, tag="sumc", name="sumc")
            rinvc = bpool.tile([4, N], F32, tag="rinvc", name="rinvc")
            rinvcb = bpool.tile([4, N], BF16, tag="rinvcb", name="rinvcb")
            nc.scalar.dma_start(sumc[:], r4[:, 0, c, :])
            nc.vector.reciprocal_approx_fast(rinvc[:], sumc[:])
            nc.vector.tensor_copy(rinvcb[:], rinvc[:])
            nc.scalar.dma_start(st_["rinvb"][4 * c:4 * c + 4, :],
                                rinvcb[:])

        pending_out = []

        def flush_out(n):
            # out DMA deferred one batch: by issue time ysb is long done,
            # so the SP queue never blocks on DVE
            while len(pending_out) > n:
                ysb_, b__ = pending_out.pop(0)
                nc.scalar.dma_start(out[b__], ysb_[:])

        def flush_proj(n):
            while len(pending_proj) > n:
                otn_, b_ = pending_proj.pop(0)
                otn_ = list(otn_)
                ysb = bpool.tile([128, 6, N], BF16, tag="ysb", name="ysb")
                for co in range(6):
                    pp = scps.tile([128, 2, 512], F32, tag="sc", name="pp")
                    for cp in range(6):
                        nc.tensor.matmul(
                            pp[:, 0, :N],
                            lhsT=wp[cp][:, co * 128:(co + 1) * 128],
                            rhs=otn_[cp][:], start=(cp == 0), stop=(cp == 5))
                    nc.vector.tensor_tensor(
                        out=ysb[:, co, :], in0=pp[:, 0, :N],
                        in1=pbias_t[:, co:co + 1].broadcast_to([128, N]),
                        op=OP.add)
                pending_out.append((ysb, b_))
                flush_out(1)

        batch_state = {}
        strip_pts = {}

        def emit_scores(b, p):
            # scores + exp for strip p; the AV consuming these pts is
            # emitted one unit later so the PE has ready score matmuls to
            # chew on while ScalarE works through the exps
            if p == 0:
                rbuf = bpool.tile([128, 3 * N], F32, tag="rowbuf",
                                  name="rowbuf")
                rinvb = bpool.tile([12, N], BF16, tag="rinvb",
                                   name="rinvb")
                # unwritten chunks must be finite: the rb matmul contracts
                # all 12 rows (zeros in sel, but 0*NaN = NaN)
                nc.vector.memset(rinvb[:], 0.0)
                batch_state[b] = dict(rowbuf=rbuf, otsb=[None] * 6,
                                      otn=[None] * 6, rinvb=rinvb)
            # both heads in one 2-bank sc tile: the tile_position-paired
            # score matmuls only co-execute on the PE when they share the
            # accumulation tile
            pts = []
            for (r0, rows) in kblocks:
                sc = scps.tile([128, 2, 512], F32, tag="sc")
                kc = b * N + r0
                for h in range(2):
                    nc.tensor.matmul(
                        sc[:rows, h, :N],
                        lhsT=kT[h * D:(h + 1) * D, p, kc:kc + rows],
                        rhs=qT[h * D:(h + 1) * D, p, b * N:(b + 1) * N],
                        start=True, stop=True,
                        tile_position=(h * D, 0))
                pt = ptpool.tile([128, 2, N], BF16, tag="pt")
                nc.scalar.activation(pt[:rows, :, :], sc[:rows, :, :N],
                                     AF.Exp)
                pts.append(pt)
            strip_pts[(b, p)] = pts

        def emit_av(b, p):
            st_ = batch_state[b]
            rowbuf, otsb = st_["rowbuf"], st_["otsb"]
            pts = strip_pts.pop((b, p))
            osb = otsbp.tile([128, N], BF16, tag="otsb")
            for h in range(2):
                hh = 2 * p + h
                ot = otps.tile([128, 512], F32, tag="ot")
                for ik, (r0, rows) in enumerate(kblocks):
                    nc.tensor.matmul(
                        ot[:D + 1, :N],
                        lhsT=v65[(b, ik)][:rows, hh, :],
                        rhs=pts[ik][:rows, h, :],
                        start=(ik == 0), stop=(ik == len(kblocks) - 1))
                # sum row -> stride-32 partition, free chunk hh//4;
                # AV evac split across ACT / DVE to balance engines
                if h == 0:
                    nc.scalar.copy(
                        rowbuf[(hh % 4) * 32:(hh % 4) * 32 + 1,
                               (hh // 4) * N:(hh // 4 + 1) * N],
                        ot[D:D + 1, :N])
                    nc.scalar.copy(osb[h * D:(h + 1) * D, :], ot[:D, :N])
                else:
                    nc.vector.tensor_copy(
                        rowbuf[(hh % 4) * 32:(hh % 4) * 32 + 1,
                               (hh // 4) * N:(hh // 4 + 1) * N],
                        ot[D:D + 1, :N])
                    nc.vector.tensor_copy(osb[h * D:(h + 1) * D, :],
                                          ot[:D, :N])
            otsb[p] = osb
            if p % 2 == 1:
                # strips 2c, 2c+1 done: kick their sums chunk now, and
                # normalize the PREVIOUS pair (its rinvb had a full unit
                # of slack)
                c = p // 2
                gather_chunk(b, c)
                if c > 0:
                    norm_pair(b, c - 1)
            elif p == 0:
                # previous batch's proj (otn complete since its tail)
                flush_proj(0)

        def emit_tail(b):
            # last pair's normalize, then queue the proj
            norm_pair(b, 2)
            st_ = batch_state.pop(b)
            pending_proj.append((st_["otn"], b))

        # interleave phase B at sub-strip granularity, software-pipelined:
        # scores(p+1) sit between scores(p) and AV(p) so AV's exp inputs
        # are ready when the PE reaches it (no p-state reset per strip)
        seq = [("S", 0), ("S", 1), ("A", 0), ("S", 2), ("A", 1), ("S", 3),
               ("A", 2), ("S", 4), ("A", 3), ("S", 5), ("A", 4), ("A", 5),
               ("T", 0)]
        units = [(b, k, p) for b in range(BPC) for (k, p) in seq]
        ucur = 0

        def unit_allowed(b, i):
            return (N * (b + 1) + 127) // 128 - 1 <= i - 2

        def emit_unit(b, k, p):
            if k == "S":
                emit_scores(b, p)
            elif k == "A":
                emit_av(b, p)
            else:
                emit_tail(b)

        def pump(upto, i):
            nonlocal ucur
            while ucur < min(upto, len(units)) \
                    and unit_allowed(units[ucur][0], i):
                emit_unit(*units[ucur])
                ucur += 1

        for i in range(NT):
            st = start_tile(i)
            base = ucur
            target = max(0, (i - 3) * len(units) // (NT - 4))
            quota = max(0, target - base)
            # zipper phase-B units between the qkv matmul groups so the
            # PE stream stays gapless
            for step in range(4):
                if step < 3:
                    tile_mm(st, (2 * step, 2 * step + 1))
                else:
                    tile_post(st)
                pump(base + quota * (step + 1) // 4, i)
        while ucur < len(units):
            emit_unit(*units[ucur])
            ucur += 1
        flush_proj(0)
        flush_out(0)

    nc.compile()
    return nc


def _host_tables(rope_tensor, qn_g, qn_b, kn_g, kn_b, P, L):
    """Fused rope+LN tables [6, TPAD, 64]: qCT,qST,qB2,kCT,kST,kB2."""
    n_img = N - P - L
    rt = np.asarray(rope_tensor, np.float64)
    cos = rt[:n_img, :, 0]
    sin = rt[:n_img, :, 1]
    c_full = np.ones((N, D // 2))
    s_full = np.zeros((N, D // 2))
    c_full[P:N - L] = cos
    s_full[P:N - L] = sin
    reps = TPAD // N + 2
    c_all = np.tile(c_full, (reps, 1))[:TPAD]
    s_all = np.tile(s_full, (reps, 1))[:TPAD]
    c_all[T:] = 1.0
    s_all[T:] = 0.0

    def mk(g, b):
        # deinterleaved layout: cols [0:32] = even lanes, [32:64] = odd
        g = np.asarray(g, np.float64)
        b = np.asarray(b, np.float64)
        ge, go = g[0::2], g[1::2]
        be, bo = b[0::2], b[1::2]
        CT = np.empty((TPAD, D))
        ST = np.empty((TPAD, D))
        B2 = np.empty((TPAD, D))
        CT[:, 0:32] = ge[None, :] * c_all
        CT[:, 32:64] = go[None, :] * c_all
        # STs2 layout for the swap-free combine: m2 = z*ST, then
        # rot_e = m1_e + m2_o (needs m2_o = -zO*go*s) and
        # rot_o = m1_o + m2_e (needs m2_e = +zE*ge*s)
        ST[:, 0:32] = ge[None, :] * s_all
        ST[:, 32:64] = -(go[None, :] * s_all)
        B2[:, 0:32] = be[None, :] * c_all - bo[None, :] * s_all
        B2[:, 32:64] = bo[None, :] * c_all + be[None, :] * s_all
        return CT, ST, B2

    qsc = 1.0 / np.sqrt(D)
    qCT, qST, qB2 = mk(np.asarray(qn_g, np.float64) * qsc,
                       np.asarray(qn_b, np.float64) * qsc)
    kCT, kST, kB2 = mk(kn_g, kn_b)
    return np.stack([qCT, qST, qB2, kCT, kST, kB2]).astype(BF)


def _host_wqkv(qkv_w):
    """wqkvT [C, 6*CHW]: 6 chunks of 6 heads x 64 cols.

    q/k heads get mean-centering folded in (W' = W - per-head row mean --
    exact: the head-mean of q is linear in x) and their columns permuted
    to the deinterleaved rope-pair layout ([evens, odds]); dot products
    over d are invariant since q and k get the same permutation.  v heads
    stay in natural order.
    """
    wT = np.asarray(qkv_w, np.float64).T  # [C, 3C]
    deint = np.concatenate([np.arange(0, D, 2), np.arange(1, D, 2)])
    outw = np.empty((C, 6 * CHW), np.float64)
    for j in range(6):
        cols = wT[:, j * 384:(j + 1) * 384].reshape(C, 6, D)
        if j < 4:  # q, k: fold centering, then deinterleave
            cols = cols - cols.mean(axis=2, keepdims=True)
            cols = cols[:, :, deint]
        outw[:, j * CHW:(j + 1) * CHW] = cols.reshape(C, 384)
    return outw.astype(BF)


def _host_sel():
    s = np.zeros((12, C), np.float32)
    for k in range(12):
        s[k, k * D:(k + 1) * D] = 1.0
    return s.astype(BF)


def _make_in_maps(x, rope_tensor, qkv_w, proj_w, proj_b, qn_g, qn_b,
                  kn_g, kn_b, P, L):
    tabs = _host_tables(rope_tensor, qn_g, qn_b, kn_g, kn_b, P, L)
    wqkvT = _host_wqkv(qkv_w)
    wprojT = np.ascontiguousarray(
        np.asarray(proj_w, np.float32).T).astype(BF)
    pb = np.ascontiguousarray(np.asarray(proj_b, np.float32))
    sel = _host_sel()
    in_maps = []
    for core in range(NCORES):
        xc = x[core * BPC:(core + 1) * BPC].reshape(T, C)
        xTc = np.zeros((C, TPAD), np.float32)
        xTc[:, :T] = xc.T
        # block-major: xb[i, p, c*128+t] = xT[c*128+p, i*128+t]
        xbc = np.ascontiguousarray(
            xTc.reshape(6, 128, NT, 128).transpose(2, 1, 0, 3).reshape(
                NT, 128, 6 * 128)).astype(BF)
        in_maps.append({"xb": xbc, "wqkvT": wqkvT, "wprojT": wprojT,
                        "pbias": pb, "tabs": tabs, "sel": sel})
    return in_maps


def kernel(x, rope_tensor, qkv_w, proj_w, proj_b, qn_g, qn_b, kn_g, kn_b,
           num_prefix_tokens, num_latent_tokens, _spmd_kwargs=None):
    P = int(num_prefix_tokens)
    L = int(num_latent_tokens)
    x = np.asarray(x, np.float32)
    assert x.shape == (B, N, C), x.shape

    if "nc" not in _CACHE:
        _CACHE["nc"] = _build_program()
    nc = _CACHE["nc"]

    in_maps = _make_in_maps(x, rope_tensor, qkv_w, proj_w, proj_b,
                            qn_g, qn_b, kn_g, kn_b, P, L)
    res = run_bass_kernel_spmd(nc, in_maps, core_ids=list(range(NCORES)),
                               **(_spmd_kwargs or {}))
    outs = []
    for core in range(NCORES):
        # out[b, p, co, n] = y[b, n, co*128+p]
        yc = np.asarray(res.results[core]["out"], BF).astype(np.float32)
        outs.append(yc.transpose(0, 3, 2, 1).reshape(BPC, N, C))
    full = np.concatenate(outs, axis=0).astype(np.float32)
    if _spmd_kwargs is not None:
        _CACHE["last_results"] = res
    return full



# revision 89
# speedup vs baseline: 1.1613x; 1.1613x over previous
"""Trainium2 Bass kernel for nn_Attention (B=64, N=289, C=768, H=12).

Data-parallel over batch: 8 batches per NeuronCore x 8 cores, no collectives.

Per-core pipeline (all matmuls bf16 with f32 PSUM accumulation):
  1. qkv GEMM token-major.  Mean-centering of q/k is folded into the
     weights host-side (W' = W - per-head row mean, exact), so the GEMM
     directly emits zero-mean q/k and no mean stats are needed.  v heads
     are computed in the same per-block pass and scattered into per-batch
     token-major v65 tiles via SBUF->SBUF DMA (arbitrary partition shift).
  2. q/k head-wise layernorm reduces to one rsqrt scale: var = sumsq/64,
     rsqrt via DVE bit-trick + Newton, z = q * s.
  3. rope fused with the LN gain/bias via host-precomputed tables; even
     lanes on DVE, odd lanes on GPSIMD.  1/sqrt(D) folded into q tables.
  4. q/k -> feature-major strips via XBAR DMA transpose ([128,128] bf16
     SBUF->SBUF), split across the two HWDGE engines (k on SP, q on ACT)
     so the two transpose pipes run in parallel.
  5. scores computed transposed ST[k_tok, q_tok] (softmax without max
     subtraction -- LN-bounded logits), exp on ScalarE straight from PSUM.
  6. AV with a ones-column appended to V: OT[65, q] where row 64 carries
     the softmax sums; sum rows gathered via aligned ScalarE copies into
     stride-32 partitions + 3 partition-matched DMAs on the ACT queue.
     The normalize tail (reciprocal, broadcast, multiply) is deferred and
     emitted interleaved with the NEXT batch's scores so the PE never
     waits on it.
  7. proj GEMM feature-major + bias, bf16 out DMA as yT[768, 2312]; the
     host transposes back and casts to f32.
"""

import sys

if "/opt/trn_rl_repo" not in sys.path:
    sys.path.insert(0, "/opt/trn_rl_repo")

from contextlib import ExitStack

import ml_dtypes
import numpy as np

import concourse.bass as bass
import concourse.tile as tile
from concourse import bacc, mybir
from concourse.bass_utils import run_bass_kernel_spmd

F32 = mybir.dt.float32
BF16 = mybir.dt.bfloat16
I32 = mybir.dt.int32
BF = ml_dtypes.bfloat16
OP = mybir.AluOpType
AF = mybir.ActivationFunctionType

B, N, C, H = 64, 289, 768, 12
D = C // H  # 64
NCORES = 8
BPC = B // NCORES  # 8 batches per core
T = BPC * N  # 2312 tokens per core
NT = (T + 127) // 128  # 19 token blocks
TPAD = NT * 128  # 2432
EPS = 1e-5
CHW = 6 * D  # 384 columns per qkv chunk (6 heads; centering is in W)
MAGIC = 0x5F3759DF

_CACHE = {}


def _batch_blocks():
    out, r = [], 0
    while r < N:
        rows = min(128, N - r)
        out.append((r, rows))
        r += rows
    return out


def _block_pieces(i):
    """Split global token block i into per-(batch, kblock) pieces.

    Returns [(src_row, b, kb, dst_row, take)].
    """
    pieces = []
    g = 128 * i
    end = min(128 * (i + 1), T)
    while g < end:
        b = g // N
        l = g - N * b
        kb = l // 128
        row0 = l - 128 * kb
        take = min(end, N * (b + 1), N * b + 128 * (kb + 1)) - g
        pieces.append((g - 128 * i, b, kb, row0, take))
        g += take
    return pieces


def _build_program():
    nc = bacc.Bacc("TRN2", target_bir_lowering=False, debug=False,
                   num_devices=NCORES)

    # block-major x: xb[i, p, c*128+t] = x^T[c*128+p, i*128+t] so each
    # 128-token block loads as one 1536B-contiguous descriptor per partition
    xb = nc.dram_tensor("xb", [NT, 128, 6 * 128], BF16,
                        kind="ExternalInput").ap()
    wqkvT = nc.dram_tensor("wqkvT", [C, 6 * CHW], BF16,
                           kind="ExternalInput").ap()
    wprojT = nc.dram_tensor("wprojT", [C, C], BF16, kind="ExternalInput").ap()
    pbias = nc.dram_tensor("pbias", [C], F32, kind="ExternalInput").ap()
    # fused rope+LN tables, order: qCT, qST, qB2, kCT, kST, kB2
    tabs = nc.dram_tensor("tabs", [6, TPAD, D], BF16,
                          kind="ExternalInput").ap()
    sel = nc.dram_tensor("sel", [12, C], BF16, kind="ExternalInput").ap()
    # batch-contiguous out: out[b, p, co, n] = y[b, n, co*128+p]; one
    # 3468B descriptor per partition per batch
    out = nc.dram_tensor("out", [BPC, 128, 6, N], BF16,
                         kind="ExternalOutput").ap()

    with tile.TileContext(nc) as tc, ExitStack() as ctx:
        consts = ctx.enter_context(tc.tile_pool(name="consts", bufs=1))
        work = ctx.enter_context(tc.tile_pool(name="work", bufs=2))
        blk = ctx.enter_context(tc.tile_pool(name="blk", bufs=2))
        shal = ctx.enter_context(tc.tile_pool(name="shal", bufs=2))
        rope_scr = ctx.enter_context(tc.tile_pool(name="rope", bufs=2))
        strips = ctx.enter_context(tc.tile_pool(name="strips", bufs=1))
        vpool = ctx.enter_context(tc.tile_pool(name="vpool", bufs=15))
        ptpool = ctx.enter_context(tc.tile_pool(name="ptpool", bufs=7))
        otsbp = ctx.enter_context(tc.tile_pool(name="otsbp", bufs=11))
        otnpool = ctx.enter_context(tc.tile_pool(name="otnpool", bufs=10))
        bpool = ctx.enter_context(tc.tile_pool(name="bpool", bufs=2))
        mmps = ctx.enter_context(tc.tile_pool(name="mmps", bufs=2,
                                              space="PSUM"))
        scps = ctx.enter_context(tc.tile_pool(name="scps", bufs=2,
                                              space="PSUM"))
        otps = ctx.enter_context(tc.tile_pool(name="otps", bufs=2,
                                              space="PSUM"))

        # ---- persistent constants ----
        # xg block 0/1 prefetches go first on SP (see xgpool below), then
        # wq (block-0 matmuls pace with their arrival); tabs first on ACT
        # (block-0 rope needs them), then wp/sel/pbias (phase B only).
        xgpool = ctx.enter_context(tc.tile_pool(name="xgpool", bufs=3))
        xg_tiles = {}

        def fetch_xg(i):
            if i < NT and i not in xg_tiles:
                t = xgpool.tile([128, 6, 128], BF16, tag="xg")
                nc.scalar.dma_start(
                    t[:], xb[i].rearrange("p (c t) -> p c t", t=128))
                xg_tiles[i] = t
            return xg_tiles.get(i)

        fetch_xg(0)
        fetch_xg(1)
        # DMA queues fair-share bandwidth, so anything issued at t=0
        # delays the critical wq+xg path.  Only wq goes out up front;
        # tabs are issued after block 0 and wp/sel/pbias after block 2
        # (see emit_tile) -- by then the startup burst has drained.
        wq = []
        for c in range(6):
            t = consts.tile([128, 6 * CHW], BF16, tag=f"wq{c}")
            dq = nc.sync if c % 2 == 0 else nc.scalar
            dq.dma_start(t[:], wqkvT[c * 128:(c + 1) * 128, :])
            wq.append(t)
        tab_t = [consts.tile([128, NT, D], BF16, tag=f"tab{k}",
                             name=f"tab{k}") for k in range(6)]
        wp = [consts.tile([128, C], BF16, tag=f"wp{c}", name=f"wp{c}")
              for c in range(6)]
        pbias_t = consts.tile([128, 6], F32, tag="pbias")
        sel_t = consts.tile([12, C], BF16, tag="sel")
        magic_t = consts.tile([128, 24], I32, tag="magic")
        nc.vector.memset(magic_t[:], MAGIC)

        def load_tabs():
            # emitted between block-0's evacs and its rope: the ACT
            # sequencer issues these only once the evacs retire, by which
            # time the wq burst has mostly drained
            for k in range(6):
                nc.scalar.dma_start(
                    tab_t[k][:],
                    tabs[k].rearrange("(i p) d -> p i d", p=128))

        def load_phaseb_consts():
            for c in range(6):
                nc.scalar.dma_start(
                    wp[c][:], wprojT[c * 128:(c + 1) * 128, :])
            nc.scalar.dma_start(
                pbias_t[:], pbias.rearrange("(a p) -> p a", p=128))
            nc.scalar.dma_start(sel_t[:], sel)

        # q/k feature-major strips, all head pairs in one tile: strip p
        # lives at [:, p, :].  One XBAR DMA transpose per (block, half)
        # fills all six strips (out[:, j, :] = in[:, 128j:128j+128].T).
        qT = strips.tile([128, 6, TPAD], BF16, tag="qT", name="qT")
        kT = strips.tile([128, 6, TPAD], BF16, tag="kT", name="kT")

        kblocks = _batch_blocks()
        v65 = {}  # (b, kb) -> tile [128, H, D+1]

        def get_v65(b, kb):
            if (b, kb) not in v65:
                # ones column arrives via the vsb scatter; no memset needed
                t = vpool.tile([128, H, D + 1], BF16, tag="v65")
                v65[(b, kb)] = t
            return v65[(b, kb)]

        # ---- phase A: qkv GEMM + LN + rope + DMA transpose, per 128-blk.
        # Split into start / per-chunk-pair matmuls / post so phase-B units
        # can zipper between the matmul groups, keeping the PE stream
        # gapless (the p-state ramp only holds at 2.4GHz while busy).
        def start_tile(i):
            xg = fetch_xg(i)
            fetch_xg(i + 2)  # prefetch ahead of this block's transposes
            xg_tiles.pop(i, None)
            sumsq = work.tile([128, 24], F32, tag="sumsq")
            qsb = blk.tile([128, H, D], BF16, tag="qsb")
            ksb = blk.tile([128, H, D], BF16, tag="ksb")
            # vsb [128, H, 65]: ones column baked in so scatter rows are
            # contiguous
            vsb = blk.tile([128, H, D + 1], BF16, tag="vsb")
            nc.vector.memset(vsb[:, :, D:D + 1], 1.0)
            return dict(i=i, xg=xg, sumsq=sumsq, qsb=qsb, ksb=ksb, vsb=vsb)

        def tile_mm(st, js):
            # qkv GEMM chunks; evacuation trails on ScalarE (the DVE queue
            # is busy with the rope chain, and mmps recycling gates the
            # next matmul group)
            xg, qsb, ksb, vsb = st["xg"], st["qsb"], st["ksb"], st["vsb"]
            halves = (qsb, ksb)
            for j in js:
                ps = mmps.tile([128, CHW], F32, tag="mm")
                for c in range(6):
                    nc.tensor.matmul(ps[:], lhsT=xg[:, c, :],
                                     rhs=wq[c][:, j * CHW:(j + 1) * CHW],
                                     start=(c == 0), stop=(c == 5))
                if j < 4:
                    dst = halves[j // 2]
                    jl = j % 2
                    nc.scalar.copy(
                        dst[:, jl * 6:(jl + 1) * 6, :].rearrange(
                            "p h d -> p (h d)"), ps[:])
                else:
                    nc.scalar.copy(
                        vsb[:, (j - 4) * 6:(j - 3) * 6, :D],
                        ps[:].rearrange("p (h d) -> p h d", d=D))

        def tile_post(st):
            i = st["i"]
            sumsq, vsb = st["sumsq"], st["vsb"]
            halves = (st["qsb"], st["ksb"])
            if i == 0:
                load_tabs()
            elif i == 3:
                load_phaseb_consts()

            # squares + per-head reduces (ScalarE + DVE); the square scratch
            # borrows the m1 rope tag (freed by its reduce before the rope
            # chain reuses the buffer)
            for half in range(2):
                sq = rope_scr.tile([128, H, D], BF16, tag="m1")
                nc.scalar.activation(sq[:], halves[half][:], AF.Square)
                nc.vector.tensor_reduce(
                    out=sumsq[:, half * 12:(half + 1) * 12], in_=sq[:],
                    axis=mybir.AxisListType.X, op=OP.add)

            # one consolidated stats chain on [128, 24]:
            # s = rsqrt(sumsq/64 + eps)  (q/k are zero-mean by construction)
            vpe = shal.tile([128, 24], F32, tag="vpe")
            srs = work.tile([128, 24], F32, tag="srs")
            nt1 = shal.tile([128, 24], F32, tag="nt1")
            nc.vector.tensor_scalar(out=vpe[:], in0=sumsq[:],
                                    scalar1=1.0 / D, scalar2=float(EPS),
                                    op0=OP.mult, op1=OP.add)
            nc.vector.tensor_scalar(out=srs[:].bitcast(I32),
                                    in0=vpe[:].bitcast(I32), scalar1=1,
                                    scalar2=None,
                                    op0=OP.logical_shift_right)
            nc.vector.tensor_tensor(out=srs[:].bitcast(I32),
                                    in0=magic_t[:],
                                    in1=srs[:].bitcast(I32),
                                    op=OP.subtract)
            # Newton: y *= 1.5 - 0.5*x*y^2
            nc.vector.tensor_mul(nt1[:], srs[:], srs[:])
            nc.vector.tensor_mul(nt1[:], nt1[:], vpe[:])
            nc.vector.tensor_scalar(out=nt1[:], in0=nt1[:],
                                    scalar1=-0.5, scalar2=1.5,
                                    op0=OP.mult, op1=OP.add)
            nc.vector.tensor_mul(srs[:], srs[:], nt1[:])
            # bf16 copy: an f32 broadcast operand halves DVE throughput
            srsb = work.tile([128, 24], BF16, tag="srsb")
            nc.vector.tensor_copy(srsb[:], srs[:])

            for half, (tb, rot_tag, dst, dq) in enumerate(
                    ((0, "qrot", qT, nc.sync), (3, "krot", kT, nc.sync))):
                hsb = halves[half]
                hsrs = srsb[:, half * 12:(half + 1) * 12]
                z = blk.tile([128, H, D], BF16, tag="z" + rot_tag)
                # z = q * s with s broadcast along d (q is zero-mean)
                nc.vector.tensor_tensor(
                    out=z[:], in0=hsb[:],
                    in1=hsrs[:, :, None].broadcast_to([128, H, D]),
                    op=OP.mult)

                # rope (+ folded gain/bias), deinterleaved pair layout:
                # head cols [0:32] = even lanes, [32:64] = odd lanes.
                # All DVE (GpSimd concurrency poisons DVE throughput ~4x).
                # m1 = z*CT; m2 = z*STs2 with STs2 = [ge*s | -go*s], so the
                # cross-half combine is two plain-slice adds (no swap AP):
                #   rot_e = m1_e + m2_o,  rot_o = m1_o + m2_e
                rot = blk.tile([128, H, D], BF16, tag=rot_tag)
                CT = tab_t[tb][:, i, :]
                ST = tab_t[tb + 1][:, i, :]
                B2 = tab_t[tb + 2][:, i, :]

                def bcf(ap):
                    return ap[:, None, :].broadcast_to([128, H, D])

                # no in-place ops: DVE runs read-modify-write at half rate
                m1 = rope_scr.tile([128, H, D], BF16, tag="m1")
                m2 = rope_scr.tile([128, H, D], BF16, tag="m2")
                t3 = rope_scr.tile([128, H, D], BF16, tag="t3")
                nc.vector.tensor_mul(m1[:], z[:], bcf(CT))
                nc.vector.tensor_mul(m2[:], z[:], bcf(ST))
                nc.vector.tensor_add(t3[:, :, 0:32], m1[:, :, 0:32],
                                     m2[:, :, 32:64])
                nc.vector.tensor_add(t3[:, :, 32:64], m1[:, :, 32:64],
                                     m2[:, :, 0:32])
                nc.vector.tensor_add(rot[:], t3[:], bcf(B2))

                # one XBAR DMA transpose fills all 6 strips of this block
                dq.dma_start_transpose(
                    dst[:, :, i * 128:(i + 1) * 128],
                    rot[:].rearrange("p h d -> p (h d)"))

            # v scatter last: vsb is long done by the time SP reaches these
            for (src_row, b, kb, dst_row, take) in _block_pieces(i):
                vt = get_v65(b, kb)
                nc.sync.dma_start(
                    vt[dst_row:dst_row + take, :, :],
                    vsb[src_row:src_row + take, :, :])

        # ---- phase B: per batch attention + proj ----
        pending_norm = []
        pending_proj = []

        def flush_norm():
            while pending_norm:
                otsb_, rinvb, b_ = pending_norm.pop(0)
                otn = []
                for p in range(6):
                    o = otnpool.tile([128, N], BF16, tag="otn")
                    # one matmul broadcasts both heads' reciprocal rows
                    # across the pair's 128 partitions; lives in scps so
                    # the AV ot ring never waits on the otn chain
                    rb = scps.tile([128, 2, 512], F32, tag="sc")
                    nc.tensor.matmul(rb[:, 0, :N],
                                     lhsT=sel_t[:, p * 128:(p + 1) * 128],
                                     rhs=rinvb[:], start=True, stop=True)
                    nc.vector.tensor_mul(o[:], otsb_[p][:], rb[:, 0, :N])
                    otn.append(o)
                pending_proj.append((otn, b_))

        pending_out = []

        def flush_out(n):
            # out DMA deferred one batch: by issue time ysb is long done,
            # so the SP queue never blocks on DVE
            while len(pending_out) > n:
                ysb_, b__ = pending_out.pop(0)
                nc.scalar.dma_start(out[b__], ysb_[:])

        def flush_proj(n):
            while len(pending_proj) > n:
                otn_, b_ = pending_proj.pop(0)
                otn_ = list(otn_)
                ysb = bpool.tile([128, 6, N], BF16, tag="ysb", name="ysb")
                for co in range(6):
                    pp = scps.tile([128, 2, 512], F32, tag="sc", name="pp")
                    for cp in range(6):
                        nc.tensor.matmul(
                            pp[:, 0, :N],
                            lhsT=wp[cp][:, co * 128:(co + 1) * 128],
                            rhs=otn_[cp][:], start=(cp == 0), stop=(cp == 5))
                    nc.vector.tensor_tensor(
                        out=ysb[:, co, :], in0=pp[:, 0, :N],
                        in1=pbias_t[:, co:co + 1].broadcast_to([128, N]),
                        op=OP.add)
                pending_out.append((ysb, b_))
                flush_out(1)

        batch_state = {}
        strip_pts = {}

        def emit_scores(b, p):
            # scores + exp for strip p; the AV consuming these pts is
            # emitted one unit later so the PE has ready score matmuls to
            # chew on while ScalarE works through the exps
            if p == 0:
                rbuf = bpool.tile([128, 3 * N], F32, tag="rowbuf",
                                  name="rowbuf")
                batch_state[b] = (rbuf, [None] * 6)
            # both heads in one 2-bank sc tile: the tile_position-paired
            # score matmuls only co-execute on the PE when they share the
            # accumulation tile
            pts = []
            for (r0, rows) in kblocks:
                sc = scps.tile([128, 2, 512], F32, tag="sc")
                kc = b * N + r0
                for h in range(2):
                    nc.tensor.matmul(
                        sc[:rows, h, :N],
                        lhsT=kT[h * D:(h + 1) * D, p, kc:kc + rows],
                        rhs=qT[h * D:(h + 1) * D, p, b * N:(b + 1) * N],
                        start=True, stop=True,
                        tile_position=(h * D, 0))
                pt = ptpool.tile([128, 2, N], BF16, tag="pt")
                nc.scalar.activation(pt[:rows, :, :], sc[:rows, :, :N],
                                     AF.Exp)
                pts.append(pt)
            strip_pts[(b, p)] = pts

        def emit_av(b, p):
            rowbuf, otsb = batch_state[b]
            pts = strip_pts.pop((b, p))
            osb = otsbp.tile([128, N], BF16, tag="otsb")
            for h in range(2):
                hh = 2 * p + h
                ot = otps.tile([128, 512], F32, tag="ot")
                for ik, (r0, rows) in enumerate(kblocks):
                    nc.tensor.matmul(
                        ot[:D + 1, :N],
                        lhsT=v65[(b, ik)][:rows, hh, :],
                        rhs=pts[ik][:rows, h, :],
                        start=(ik == 0), stop=(ik == len(kblocks) - 1))
                # sum row -> stride-32 partition, free chunk hh//4;
                # AV evac split across ACT / DVE to balance engines
                if h == 0:
                    nc.scalar.copy(
                        rowbuf[(hh % 4) * 32:(hh % 4) * 32 + 1,
                               (hh // 4) * N:(hh // 4 + 1) * N],
                        ot[D:D + 1, :N])
                    nc.scalar.copy(osb[h * D:(h + 1) * D, :], ot[:D, :N])
                else:
                    nc.vector.tensor_copy(
                        rowbuf[(hh % 4) * 32:(hh % 4) * 32 + 1,
                               (hh // 4) * N:(hh // 4 + 1) * N],
                        ot[D:D + 1, :N])
                    nc.vector.tensor_copy(osb[h * D:(h + 1) * D, :],
                                          ot[:D, :N])
            otsb[p] = osb
            if p == 1:
                # previous batch: rb broadcast + otn muls (DVE) first
                # (a unit later than the tail so the gather/recip chain
                # has slack) ...
                flush_norm()
            elif p == 2:
                # ... then its proj a unit later, once otn is long ready
                flush_proj(0)

        def emit_tail(b):
            rowbuf, otsb = batch_state.pop(b)
            # gather + reciprocal now, so rinvb is long ready by the time
            # the deferred rb/otn (flush_norm) hits the PE next batch
            sums_sb = bpool.tile([12, N], F32, tag="sums_sb")
            r4 = rowbuf[:].rearrange("(a b) (c n) -> a b c n", b=32, n=N)
            for c in range(3):
                nc.scalar.dma_start(sums_sb[4 * c:4 * c + 4, :],
                                    r4[:, 0, c, :])
            rinv = bpool.tile([12, N], F32, tag="rinv")
            rinvb = bpool.tile([12, N], BF16, tag="rinvb")
            nc.vector.reciprocal_approx_fast(rinv[:], sums_sb[:])
            nc.vector.tensor_copy(rinvb[:], rinv[:])
            pending_norm.append((otsb, rinvb, b))

        # interleave phase B at sub-strip granularity, software-pipelined:
        # scores(p+1) sit between scores(p) and AV(p) so AV's exp inputs
        # are ready when the PE reaches it (no p-state reset per strip)
        seq = [("S", 0), ("S", 1), ("A", 0), ("S", 2), ("A", 1), ("S", 3),
               ("A", 2), ("S", 4), ("A", 3), ("S", 5), ("A", 4), ("A", 5),
               ("T", 0)]
        units = [(b, k, p) for b in range(BPC) for (k, p) in seq]
        ucur = 0

        def unit_allowed(b, i):
            return (N * (b + 1) + 127) // 128 - 1 <= i - 2

        def emit_unit(b, k, p):
            if k == "S":
                emit_scores(b, p)
            elif k == "A":
                emit_av(b, p)
            else:
                emit_tail(b)

        def pump(upto, i):
            nonlocal ucur
            while ucur < min(upto, len(units)) \
                    and unit_allowed(units[ucur][0], i):
                emit_unit(*units[ucur])
                ucur += 1

        for i in range(NT):
            st = start_tile(i)
            base = ucur
            target = max(0, (i - 3) * len(units) // (NT - 4))
            quota = max(0, target - base)
            # zipper phase-B units between the qkv matmul groups so the
            # PE stream stays gapless
            for step in range(4):
                if step < 3:
                    tile_mm(st, (2 * step, 2 * step + 1))
                else:
                    tile_post(st)
                pump(base + quota * (step + 1) // 4, i)
        while ucur < len(units):
            emit_unit(*units[ucur])
            ucur += 1
        flush_norm()
        flush_proj(0)
        flush_out(0)

    nc.compile()
    return nc


def _host_tables(rope_tensor, qn_g, qn_b, kn_g, kn_b, P, L):
    """Fused rope+LN tables [6, TPAD, 64]: qCT,qST,qB2,kCT,kST,kB2."""
    n_img = N - P - L
    rt = np.asarray(rope_tensor, np.float64)
    cos = rt[:n_img, :, 0]
    sin = rt[:n_img, :, 1]
    c_full = np.ones((N, D // 2))
    s_full = np.zeros((N, D // 2))
    c_full[P:N - L] = cos
    s_full[P:N - L] = sin
    reps = TPAD // N + 2
    c_all = np.tile(c_full, (reps, 1))[:TPAD]
    s_all = np.tile(s_full, (reps, 1))[:TPAD]
    c_all[T:] = 1.0
    s_all[T:] = 0.0

    def mk(g, b):
        # deinterleaved layout: cols [0:32] = even lanes, [32:64] = odd
        g = np.asarray(g, np.float64)
        b = np.asarray(b, np.float64)
        ge, go = g[0::2], g[1::2]
        be, bo = b[0::2], b[1::2]
        CT = np.empty((TPAD, D))
        ST = np.empty((TPAD, D))
        B2 = np.empty((TPAD, D))
        CT[:, 0:32] = ge[None, :] * c_all
        CT[:, 32:64] = go[None, :] * c_all
        # STs2 layout for the swap-free combine: m2 = z*ST, then
        # rot_e = m1_e + m2_o (needs m2_o = -zO*go*s) and
        # rot_o = m1_o + m2_e (needs m2_e = +zE*ge*s)
        ST[:, 0:32] = ge[None, :] * s_all
        ST[:, 32:64] = -(go[None, :] * s_all)
        B2[:, 0:32] = be[None, :] * c_all - bo[None, :] * s_all
        B2[:, 32:64] = bo[None, :] * c_all + be[None, :] * s_all
        return CT, ST, B2

    qsc = 1.0 / np.sqrt(D)
    qCT, qST, qB2 = mk(np.asarray(qn_g, np.float64) * qsc,
                       np.asarray(qn_b, np.float64) * qsc)
    kCT, kST, kB2 = mk(kn_g, kn_b)
    return np.stack([qCT, qST, qB2, kCT, kST, kB2]).astype(BF)


def _host_wqkv(qkv_w):
    """wqkvT [C, 6*CHW]: 6 chunks of 6 heads x 64 cols.

    q/k heads get mean-centering folded in (W' = W - per-head row mean --
    exact: the head-mean of q is linear in x) and their columns permuted
    to the deinterleaved rope-pair layout ([evens, odds]); dot products
    over d are invariant since q and k get the same permutation.  v heads
    stay in natural order.
    """
    wT = np.asarray(qkv_w, np.float64).T  # [C, 3C]
    deint = np.concatenate([np.arange(0, D, 2), np.arange(1, D, 2)])
    outw = np.empty((C, 6 * CHW), np.float64)
    for j in range(6):
        cols = wT[:, j * 384:(j + 1) * 384].reshape(C, 6, D)
        if j < 4:  # q, k: fold centering, then deinterleave
            cols = cols - cols.mean(axis=2, keepdims=True)
            cols = cols[:, :, deint]
        outw[:, j * CHW:(j + 1) * CHW] = cols.reshape(C, 384)
    return outw.astype(BF)


def _host_sel():
    s = np.zeros((12, C), np.float32)
    for k in range(12):
        s[k, k * D:(k + 1) * D] = 1.0
    return s.astype(BF)


def _make_in_maps(x, rope_tensor, qkv_w, proj_w, proj_b, qn_g, qn_b,
                  kn_g, kn_b, P, L):
    tabs = _host_tables(rope_tensor, qn_g, qn_b, kn_g, kn_b, P, L)
    wqkvT = _host_wqkv(qkv_w)
    wprojT = np.ascontiguousarray(
        np.asarray(proj_w, np.float32).T).astype(BF)
    pb = np.ascontiguousarray(np.asarray(proj_b, np.float32))
    sel = _host_sel()
    in_maps = []
    for core in range(NCORES):
        xc = x[core * BPC:(core + 1) * BPC].reshape(T, C)
        xTc = np.zeros((C, TPAD), np.float32)
        xTc[:, :T] = xc.T
        # block-major: xb[i, p, c*128+t] = xT[c*128+p, i*128+t]
        xbc = np.ascontiguousarray(
            xTc.reshape(6, 128, NT, 128).transpose(2, 1, 0, 3).reshape(
                NT, 128, 6 * 128)).astype(BF)
        in_maps.append({"xb": xbc, "wqkvT": wqkvT, "wprojT": wprojT,
                        "pbias": pb, "tabs": tabs, "sel": sel})
    return in_maps


def kernel(x, rope_tensor, qkv_w, proj_w, proj_b, qn_g, qn_b, kn_g, kn_b,
           num_prefix_tokens, num_latent_tokens, _spmd_kwargs=None):
    P = int(num_prefix_tokens)
    L = int(num_latent_tokens)
    x = np.asarray(x, np.float32)
    assert x.shape == (B, N, C), x.shape

    if "nc" not in _CACHE:
        _CACHE["nc"] = _build_program()
    nc = _CACHE["nc"]

    in_maps = _make_in_maps(x, rope_tensor, qkv_w, proj_w, proj_b,
                            qn_g, qn_b, kn_g, kn_b, P, L)
    res = run_bass_kernel_spmd(nc, in_maps, core_ids=list(range(NCORES)),
                               **(_spmd_kwargs or {}))
    outs = []
    for core in range(NCORES):
        # out[b, p, co, n] = y[b, n, co*128+p]
        yc = np.asarray(res.results[core]["out"], BF).astype(np.float32)
        outs.append(yc.transpose(0, 3, 2, 1).reshape(BPC, N, C))
    full = np.concatenate(outs, axis=0).astype(np.float32)
    if _spmd_kwargs is not None:
        _CACHE["last_results"] = res
    return full

